# revision 18
# baseline (speedup 1.0000x reference)
"""Multi-head attention (B=4, S=2048, E=1024, H=16, D=64) on 8 TRN2 cores.

Sharding: core c handles batch b = c//2, query half = c%2 (1024 queries).
Each core computes K/V over its batch's full sequence (duplicated between the
two half-cores of a batch -- cheaper at these sizes than any collective),
attention for all 16 heads over its 1024 queries, and the output projection
for its output chunk. Outputs are disjoint -> host gather is concatenation.

The host rotates each core's sequence so its query block is always rows
0:1024 (attention is permutation-invariant over keys), and pre-transposes the
weights and activations (pure layout prep) so the e-contraction
projections have e on partitions.

Precision: float32r (TF32-like, full PE rate) for projections / scores /
out-projection AND the probs/V matmul; exp and accumulations in fp32.
"""

from contextlib import ExitStack

import numpy as np

import concourse.bass as bass
import concourse.tile as tile
from concourse import bacc, mybir
from concourse.bass_utils import run_bass_kernel_spmd

dt = mybir.dt
AF = mybir.ActivationFunctionType

B, S, E, H, D = 4, 2048, 1024, 16, 64
N_CORES = 8
SQ = 1024          # queries per core
P = 128
EC = E // P        # 8 e-chunks
TC = S // P        # 16 t-chunks (keys)
QC = SQ // P       # 8 query chunks
HP = H // 2        # 8 head-pairs


DEBUG = False
PV_DT = "float32r"  # dtype for the probs/V matmul operands


def _emit(nc, tc, xt_d, wqt, wkt, wvt, wot, bo, y, dbg=None):
    f32, f32r, bf16 = dt.float32, dt.float32r, dt.bfloat16
    pv_dt = getattr(dt, PV_DT)

    with ExitStack() as ctx:
        const = ctx.enter_context(tc.tile_pool(name="const", bufs=1))
        on_pool = ctx.enter_context(tc.tile_pool(name="on", bufs=1))
        ps = ctx.enter_context(tc.tile_pool(name="ps", bufs=2, space="PSUM"))
        ps_o = ctx.enter_context(
            tc.tile_pool(name="ps_o", bufs=2, space="PSUM"))

        bo_one = const.tile([1, E], f32)
        nc.sync.dma_start(bo_one[:], bo[:])
        bo_rep = const.tile([P, E], f32)
        nc.gpsimd.partition_broadcast(bo_rep[:], bo_one[:])

        ones_col = const.tile([P, 1], f32)
        nc.vector.memset(ones_col[:], 1.0)

        # normalized attention output^T: [f' (within chunk), hp, t]
        ON = on_pool.tile([P, HP, SQ], f32r)

        with ExitStack() as actx:
            xt_pool = actx.enter_context(tc.tile_pool(name="xt", bufs=1))
            w1 = actx.enter_context(tc.tile_pool(name="w1", bufs=1))
            w2 = actx.enter_context(tc.tile_pool(name="w2", bufs=2))
            ut_pool = actx.enter_context(tc.tile_pool(name="ut", bufs=4))

            # ---- phase 0: xT in SBUF (f32r); queries are cols 0:1024 ----
            # host supplies x^T; on-device we only do the f32r rounding copy
            xT = xt_pool.tile([P, EC, S], f32r)
            xt_view = xt_d.rearrange("(o p) t -> p o t", p=P)
            for tc_i in range(TC):
                x_sb = w2.tile([P, EC, P], f32, tag="xdma")
                nc.sync.dma_start(
                    x_sb[:], xt_view[:, :, tc_i * P:(tc_i + 1) * P])
                nc.vector.tensor_copy(
                    xT[:, :, tc_i * P:(tc_i + 1) * P], x_sb[:])

            if dbg is not None:
                nc.sync.dma_start(dbg["xt"], xT[:].bitcast(f32))

            # ---- per-head-pair projections + attention ----
            for hp in range(HP):
                w_sb = w1.tile([P, EC, 3, P], f32, tag="wdma")
                for wi, w_dram in enumerate((wqt, wkt, wvt)):
                    nc.sync.dma_start(
                        w_sb[:, :, wi, :],
                        w_dram.rearrange("(o p) f -> p o f", p=P)[
                            :, :, hp * P:(hp + 1) * P])
                w_r = w1.tile([P, EC, 3, P], f32r, tag="wr")
                nc.vector.tensor_copy(w_r[:], w_sb[:])

                # QT_hp [128(f), 1024(q)]; KT_hp [128(f), 2048(k)]
                qt = w1.tile([P, SQ], f32r, tag="qt")
                pq = ps.tile([P, 1024], f32, tag="S")
                for ec in range(EC):
                    for nq in range(SQ // 512):
                        nc.tensor.matmul(
                            pq[:, nq * 512:(nq + 1) * 512], w_r[:, ec, 0],
                            xT[:, ec, nq * 512:(nq + 1) * 512],
                            start=(ec == 0), stop=(ec == EC - 1))
                nc.vector.tensor_copy(qt[:], pq[:])
                kt = w1.tile([P, S], f32r, tag="kt")
                for half_k in range(2):
                    pk = ps.tile([P, 1024], f32, tag="S")
                    for ec in range(EC):
                        for nk in range(2):
                            nc.tensor.matmul(
                                pk[:, nk * 512:(nk + 1) * 512], w_r[:, ec, 1],
                                xT[:, ec, half_k * 1024 + nk * 512:
                                   half_k * 1024 + (nk + 1) * 512],
                                start=(ec == 0), stop=(ec == EC - 1))
                    nc.vector.tensor_copy(
                        kt[:, half_k * 1024:(half_k + 1) * 1024], pk[:])

                # V' [128(t), tc, 2 heads, 65] bf16 (col 64 = ones)
                vp = w1.tile([P, TC, 2, 65], pv_dt, tag="vp")
                nc.vector.tensor_copy(
                    vp[:, :, :, 64:65],
                    ones_col[:, None, None, :].to_broadcast([P, TC, 2, 1]))
                for tc_i in range(TC):
                    pv = ps.tile([P, 1024], f32, tag="S")
                    for ec in range(EC):
                        nc.tensor.matmul(
                            pv[:, :P], xT[:, ec, tc_i * P:(tc_i + 1) * P],
                            w_r[:, ec, 2],
                            start=(ec == 0), stop=(ec == EC - 1))
                    nc.vector.tensor_copy(
                        vp[:, tc_i, :, 0:64],
                        pv[:, :P].rearrange("p (h d) -> p h d", h=2))

                # scores -> exp -> (probs|1) @ V'
                po_a = ps_o.tile([65, SQ], f32, tag="po")
                po_b = ps_o.tile([65, SQ], f32, tag="po")
                for kc in range(TC):
                    for nq in range(SQ // 512):
                        sc = ps.tile([P, 1024], f32, tag="S")
                        nc.tensor.matmul(
                            sc[:, 0:512], kt[0:64, kc * P:(kc + 1) * P],
                            qt[0:64, nq * 512:(nq + 1) * 512],
                            start=True, stop=True)
                        nc.tensor.matmul(
                            sc[:, 512:1024], kt[64:128, kc * P:(kc + 1) * P],
                            qt[64:128, nq * 512:(nq + 1) * 512],
                            start=True, stop=True)
                        ut = ut_pool.tile([P, 1024], pv_dt, tag="ut")
                        nc.scalar.activation(
                            ut[:], sc[:], AF.Exp, scale=0.125)
                        nc.tensor.matmul(
                            po_a[:, nq * 512:(nq + 1) * 512], vp[:, kc, 0],
                            ut[:, 0:512],
                            start=(kc == 0), stop=(kc == TC - 1))
                        nc.tensor.matmul(
                            po_b[:, nq * 512:(nq + 1) * 512], vp[:, kc, 1],
                            ut[:, 512:1024],
                            start=(kc == 0), stop=(kc == TC - 1))

                # normalize: row 64 of po_x is the softmax denominator
                rcp_a = w1.tile([1, SQ], f32, tag="rcp_a")
                rcp_b = w1.tile([1, SQ], f32, tag="rcp_b")
                nc.vector.reciprocal(rcp_a[:], po_a[64:65, :])
                nc.vector.reciprocal(rcp_b[:], po_b[64:65, :])
                # partition_broadcast only writes correctly from base 0:
                # broadcast into full tiles, slice at read time
                brec_a = w1.tile([P, SQ], f32, tag="brec_a")
                brec_b = w1.tile([P, SQ], f32, tag="brec_b")
                nc.gpsimd.partition_broadcast(brec_a[:], rcp_a[:])
                nc.gpsimd.partition_broadcast(brec_b[:], rcp_b[:])
                if dbg is not None and hp == 0:
                    nc.sync.dma_start(dbg["qt0"], qt[:].bitcast(f32))
                    nc.sync.dma_start(dbg["kt0"], kt[:].bitcast(f32))
                nc.vector.tensor_mul(
                    ON[0:64, hp, :], po_a[0:64, :], brec_a[0:64, :])
                nc.vector.tensor_mul(
                    ON[64:128, hp, :], po_b[0:64, :], brec_b[64:128, :])
            if dbg is not None:
                nc.sync.dma_start(dbg["on"], ON[:].bitcast(f32))

        # ---- output projection: y = ON^T @ WoT + bo ----
        with ExitStack() as dctx:
            wo_pool = dctx.enter_context(tc.tile_pool(name="wo", bufs=1))
            yp = dctx.enter_context(tc.tile_pool(name="yp", bufs=2))

            wo_sb = wo_pool.tile([P, EC, E], f32)
            nc.sync.dma_start(
                wo_sb[:], wot.rearrange("(o p) f -> p o f", p=P))
            wo_r = wo_pool.tile([P, EC, E], f32r)
            nc.vector.tensor_copy(wo_r[:], wo_sb[:])
            for qc in range(QC):
                py = ps.tile([P, 1024], f32, tag="S")
                for nf in range(E // 512):
                    for hp in range(HP):
                        nc.tensor.matmul(
                            py[:, nf * 512:(nf + 1) * 512],
                            ON[:, hp, qc * P:(qc + 1) * P],
                            wo_r[:, hp, nf * 512:(nf + 1) * 512],
                            start=(hp == 0), stop=(hp == HP - 1))
                y_sb = yp.tile([P, E], f32, tag="ysb")
                nc.vector.tensor_add(y_sb[:], py[:], bo_rep[:])
                nc.sync.dma_start(y[qc * P:(qc + 1) * P, :], y_sb[:])


def _build_kernel(reps=1):
    nc = bacc.Bacc("TRN2", target_bir_lowering=False, debug=False,
                   num_devices=N_CORES)
    xt_d = nc.dram_tensor("xt", [E, S], dt.float32, kind="ExternalInput").ap()
    wqt = nc.dram_tensor("wqt", [E, E], dt.float32, kind="ExternalInput").ap()
    wkt = nc.dram_tensor("wkt", [E, E], dt.float32, kind="ExternalInput").ap()
    wvt = nc.dram_tensor("wvt", [E, E], dt.float32, kind="ExternalInput").ap()
    wot = nc.dram_tensor("wot", [E, E], dt.float32, kind="ExternalInput").ap()
    bo = nc.dram_tensor("bo", [1, E], dt.float32, kind="ExternalInput").ap()
    y = nc.dram_tensor("y", [SQ, E], dt.float32, kind="ExternalOutput").ap()

    dbg = None
    if DEBUG:
        shapes = {
            "xt": [P, EC, S], "qt0": [P, SQ], "kt0": [P, S],
            "on": [P, HP, SQ],
        }
        dbg = {k: nc.dram_tensor(f"dbg_{k}", v, dt.float32,
                                 kind="ExternalOutput").ap()
               for k, v in shapes.items()}

    with tile.TileContext(nc) as tc:
        for _ in range(reps):
            _emit(nc, tc, xt_d, wqt, wkt, wvt, wot, bo, y, dbg)
    nc.compile()
    return nc


_NC_CACHE = None


def make_in_maps(x, Wq, Wk, Wv, Wo, bo):
    x = np.asarray(x, np.float32)
    wqt = np.ascontiguousarray(np.asarray(Wq, np.float32).T)
    wkt = np.ascontiguousarray(np.asarray(Wk, np.float32).T)
    wvt = np.ascontiguousarray(np.asarray(Wv, np.float32).T)
    wot = np.ascontiguousarray(np.asarray(Wo, np.float32).T)
    bo_ = np.ascontiguousarray(np.asarray(bo, np.float32).reshape(1, E))

    in_maps = []
    for c in range(N_CORES):
        b, half = c // 2, c % 2
        # rotate so this core's query block is rows 0:SQ (keys are a
        # permutation of the sequence -- attention is invariant to key order)
        xt_rot = np.ascontiguousarray(np.roll(x[b], -half * SQ, axis=0).T)
        in_maps.append({"xt": xt_rot, "wqt": wqt, "wkt": wkt, "wvt": wvt,
                        "wot": wot, "bo": bo_})
    return in_maps


def get_nc(reps=1):
    global _NC_CACHE
    if _NC_CACHE is None:
        _NC_CACHE = {}
    if reps not in _NC_CACHE:
        _NC_CACHE[reps] = _build_kernel(reps)
    return _NC_CACHE[reps]


def kernel(x, Wq, Wk, Wv, Wo, bo):
    nc = get_nc()
    in_maps = make_in_maps(x, Wq, Wk, Wv, Wo, bo)
    res = run_bass_kernel_spmd(nc, in_maps, core_ids=list(range(N_CORES)))
    out = np.empty((B, S, E), np.float32)
    for c in range(N_CORES):
        b, half = c // 2, c % 2
        out[b, half * SQ:(half + 1) * SQ, :] = res.results[c]["y"]
    return out
